# revision 40
# baseline (speedup 1.0000x reference)
"""Trainium2 Bass kernel for nn_External_attention_44976897524182.

Math (folded):
    y      = conv1_w @ x + conv1_b                    (1x1x1 conv = channel GEMM)
    logits = lin0_w @ y ; sm = softmax(logits, axis=n)
    attn   = sm / (1e-9 + sum_k sm)
    z      = bn(conv2_w @ (lin1_w @ attn)) ; out = relu(z + x)
  Folded on host:
    A  = lin0_w @ conv1_w                 (64 x 128)
    B  = (bn_scale * conv2_w) @ lin1_w    (128 x 64)
    shift = bn_beta - bn_mean * bn_scale
  conv1's bias shifts logits per-k only; softmax over n is invariant to
  per-k shifts -> dropped. With e = exp(A@x), S[k] = sum_n e[k,n] (global),
    denom[n] = sum_k e[k,n]/S[k]
    out[c,n] = relu( (B @ (e/S))[c,n] / denom[n] + shift[c] + x[c,n] )

Sharding: 8 cores = 2 batches x 4 n-slices of 32768. Only cross-core
dependency: S (64 floats) -> AllReduce over groups [[0-3],[4-7]].

Wire format bf16 (tol 2e-2, bf16 ~4e-3). x ships c-major; the residual
(x+shift) and the output ship in the kernel's native n-major *tile* layout
(NP, 128, 8, 128) so every DMA descriptor is a contiguous 2KB run; the
host does the cheap permutations.

Phase 1: e = exp(A@x) in paired layout (128, NSH/2): partitions 0-63 even
512-tiles, 64-127 odd. 4-bank PSUM tiles -> one Exp activation per 2048
cols (ps1 is phase-scoped so phase 1 gets all 8 banks). Phase 2 per
128-position chunk: matmul [zz^T | denom] (128n x 129)
= e_chunk^T @ [B^T*invS | invS], one PSUM bank per chunk, two 4-bank
wave tensors (8 chunks in flight, a pair = exactly 2 waves) so one
strided reciprocal covers a wave's 4 denominators. The
per-position normalize+residual is a per-chunk scalar_tensor_tensor on
DVE (the bottleneck engine); relu runs on Act. All 32 xst tiles
prefetch during phase 1 + the AllReduce window.

Measured (pipelined-slope method, 8 cores): f32 baseline ~207us ->
this kernel ~115us. KABL env var enables timing ablations only.
"""

import os
import numpy as np
import ml_dtypes

_BF16 = ml_dtypes.bfloat16

_B, _C = 2, 128
_D, _H, _W = 32, 64, 64
_N = _D * _H * _W          # 131072
_NCORES = 8
_SLICES = 4
_NSH = _N // _SLICES       # 32768 per core
_K = 64
_T = 512                   # phase-1 tile width
_NT = _NSH // _T           # 64
_NP = _NT // 2             # 32 pairs
_BN_EPS = 1e-5

_nc_cache = None
last_results = None        # BassKernelResults of the most recent run


def _build(nsh=None, reps=1):
    global _nc_cache
    if nsh is None:
        nsh = _NSH
    abl = os.environ.get("KABL", "")
    full = (nsh == _NSH and reps == 1 and not abl
            and os.environ.get("KXSTB", "16") == "16")
    if full and _nc_cache is not None:
        return _nc_cache
    NSH = nsh
    NT = nsh // _T
    NP = NT // 2

    from contextlib import ExitStack
    import concourse.bass as bass  # noqa: F401
    import concourse.bacc as bacc
    import concourse.tile as tile
    import concourse.mybir as mybir

    f32 = mybir.dt.float32
    bf16 = mybir.dt.bfloat16
    AF = mybir.ActivationFunctionType
    ALU = mybir.AluOpType

    nc = bacc.Bacc(
        trn_type="TRN2",
        target_bir_lowering=False,
        debug=False,
        num_devices=_NCORES,
    )
    x_d = nc.dram_tensor("x", [_C, NSH], bf16, kind="ExternalInput").ap()
    xst_d = nc.dram_tensor("xst", [NP, _C, 8, _C], bf16,
                           kind="ExternalInput").ap()
    at_d = nc.dram_tensor("a_t", [_C, _K], bf16, kind="ExternalInput").ap()
    bt_d = nc.dram_tensor("b_t", [_K, _C], f32, kind="ExternalInput").ap()
    out_d = nc.dram_tensor("out", [NP, _C, 8, _C], bf16,
                           kind="ExternalOutput").ap()

    abl = set(abl.split(",")) if abl else set()

    with tile.TileContext(nc) as tc, ExitStack() as ctx:
        consts = ctx.enter_context(tc.tile_pool(name="consts", bufs=1))
        xpool = ctx.enter_context(tc.tile_pool(name="xpool", bufs=3))
        xstp = ctx.enter_context(tc.tile_pool(
            name="xstp", bufs=int(os.environ.get("KXSTB", "16"))))
        stp = ctx.enter_context(tc.tile_pool(name="stp", bufs=6))
        rp = ctx.enter_context(tc.tile_pool(name="rp", bufs=8))
        dram = ctx.enter_context(tc.tile_pool(name="dram", bufs=1, space="DRAM"))

        A_T = consts.tile([_C, _K], bf16)
        nc.sync.dma_start(out=A_T, in_=at_d)
        B_T = consts.tile([_K, _C], f32)
        nc.sync.dma_start(out=B_T, in_=bt_d)

        e_sb = consts.tile([_C, NSH // 2], bf16)   # paired exp values

        for _rep in range(reps):
            _emit_body(nc, tc, mybir, f32, bf16, AF, ALU, NSH, NT, NP,
                       x_d, xst_d, out_d, A_T, B_T, e_sb,
                       consts, xpool, xstp, stp, rp, dram, abl)

    nc.finalize()
    if full:
        _nc_cache = nc
    return nc


def _emit_body(nc, tc, mybir, f32, bf16, AF, ALU, NSH, NT, NP,
               x_d, xst_d, out_d, A_T, B_T, e_sb,
               consts, xpool, xstp, stp, rp, dram, abl=frozenset()):
    import concourse.bass as _bass
    NQ = NP // 4               # phase-1 quad tiles (4 pairs each)
    spart = consts.tile([_C, NQ], f32, name="spart")

    # ---- phase 1: e = exp(A@x), accumulate S. 4-bank PSUM tiles. ----
    with tc.tile_pool(name="ps1", bufs=2, space="PSUM") as ps1:
        for t in range(NQ):
            if t % 2 == 0:
                xt8 = xpool.tile([_C, 8192], bf16, tag="xt")
                nc.sync.dma_start(out=xt8,
                                  in_=x_d[:, t * 4096:(t + 2) * 4096])
            xt = xt8[:, (t % 2) * 4096:(t % 2 + 1) * 4096]
            pp = ps1.tile([_C, 2048], f32, tag="pp")
            for u in range(4):
                nc.tensor.matmul(pp[0:_K, u * _T:(u + 1) * _T],
                                 lhsT=A_T, rhs=xt[:, u * 1024:u * 1024 + _T],
                                 start=True, stop=True)
                nc.tensor.matmul(pp[_K:_C, u * _T:(u + 1) * _T],
                                 lhsT=A_T,
                                 rhs=xt[:, u * 1024 + _T:(u + 1) * 1024],
                                 start=True, stop=True, tile_position=(0, _K))
            nc.scalar.activation(out=e_sb[:, t * 2048:(t + 1) * 2048], in_=pp,
                                 func=AF.Exp, bias=0.0, scale=1.0,
                                 accum_out=spart[:, t:t + 1])

    # ---- S: reduce partials, fold halves, AllReduce over the group ----
    sred = consts.tile([_C, 1], f32)
    nc.vector.tensor_reduce(out=sred, in_=spart,
                            axis=mybir.AxisListType.X, op=ALU.add)
    shi = consts.tile([_K, 1], f32)
    nc.sync.dma_start(out=shi, in_=sred[_K:_C, :])
    s64 = consts.tile([_K, 1], f32)
    nc.vector.tensor_add(out=s64, in0=sred[0:_K, :], in1=shi)

    groups = [[0, 1, 2, 3], [4, 5, 6, 7]]
    if "nocc" in abl:
        sg = s64
    elif "ccag" in abl:
        # AllGather the 4 partial sums, reduce locally on DVE: the gather
        # rounds are pure DMA (no gpsimd compute per hop).
        cc_in = dram.tile([_K, 1], f32)
        cc_out = dram.tile([4 * _K, 1], f32)
        nc.sync.dma_start(out=cc_in, in_=s64)
        nc.gpsimd.collective_compute(
            "AllGather", ALU.bypass,
            replica_groups=groups,
            ins=[cc_in.opt()], outs=[cc_out.opt()])
        s4 = consts.tile([_K, 4], f32)
        nc.sync.dma_start(
            out=s4, in_=cc_out.opt().rearrange("(r k) o -> k (r o)", k=_K))
        sg = consts.tile([_K, 1], f32)
        nc.vector.tensor_reduce(out=sg, in_=s4,
                                axis=mybir.AxisListType.X, op=ALU.add)
    else:
        cc_in = dram.tile([_K, 1], f32)
        cc_out = dram.tile([_K, 1], f32)
        nc.sync.dma_start(out=cc_in, in_=s64)
        nc.gpsimd.collective_compute(
            "AllReduce", ALU.add,
            replica_groups=groups,
            ins=[cc_in.opt()], outs=[cc_out.opt()])
        sg = consts.tile([_K, 1], f32)
        nc.sync.dma_start(out=sg, in_=cc_out)
    invs = consts.tile([_K, 1], f32)
    nc.vector.reciprocal(out=invs, in_=sg)

    # rhs_aug = [B^T * invS | invS], duplicated into both partition halves
    rhs_aug = consts.tile([_C, _C + 1], bf16)
    nc.vector.tensor_scalar_mul(out=rhs_aug[0:_K, 0:_C], in0=B_T,
                                scalar1=invs)
    nc.vector.tensor_copy(out=rhs_aug[0:_K, _C:_C + 1], in_=invs)
    nc.sync.dma_start(out=rhs_aug[_K:_C, :], in_=rhs_aug[0:_K, :])

    # ---- phase 2 ----
    # Waves of 3 chunks; each chunk's matmul gets its own bank-aligned
    # PSUM bank, the 3 banks of a wave belong to one tensor so a single
    # strided reciprocal covers the wave's denominators. Two 3-bank
    # tensors alternate per wave.
    W = _C + 1
    nchunks = NT * 4
    xst = None
    stage = None
    wave = []          # list of (chunk_psum_ap, stage_idx, stage, xst)
    wave_id = 0

    with tc.tile_pool(name="ps2", bufs=1, space="PSUM") as ps2:
        def flush_wave():
            nonlocal wave, wave_id
            if not wave:
                return
            pbw = wave[0][0]
            nw = len(wave)
            use_recip = "trydivide" not in abl
            if use_recip:
                rq = rp.tile([_C, 4], f32, tag="rq")
                d0 = pbw[:, _C:_C + 1]
                den = _bass.AP(tensor=d0.tensor, offset=d0.offset,
                               ap=[d0.ap[0], [_T, nw]])
                if "norecip" not in abl:
                    nc.vector.reciprocal(out=rq[:, 0:nw], in_=den)
            for i, (pbw_, (e2, sidx), stg, xs, _unused) in enumerate(wave):
                if "nostt" in abl and (e2, sidx, i) != (0, 0, 0):
                    continue
                if use_recip:
                    nc.vector.scalar_tensor_tensor(
                        out=stg[:, e2, sidx, :],
                        in0=pbw_[:, i * _T:i * _T + _C],
                        scalar=rq[:, i:i + 1],
                        in1=xs[:, e2, sidx, :], op0=ALU.mult, op1=ALU.add)
                else:
                    # divide directly by the denominator column in PSUM:
                    # no reciprocal op, no rq tile at all
                    nc.vector.scalar_tensor_tensor(
                        out=stg[:, e2, sidx, :],
                        in0=pbw_[:, i * _T:i * _T + _C],
                        scalar=pbw_[:, i * _T + _C:i * _T + _C + 1],
                        in1=xs[:, e2, sidx, :],
                        op0=ALU.divide, op1=ALU.add)
            wave = []
            wave_id += 1

        for g in range(nchunks):
            q, c = g // 8, g % 8
            if c == 0 and q % 2 == 0:
                # double-pair tiles: one DMA covers pairs (q, q+1) to halve
                # HWDGE dispatch count (the real SP.SEQ bottleneck)
                xst = xstp.tile([_C, 2, 8, _C], bf16, tag="xst")
                if "noxst" not in abl:
                    nc.sync.dma_start(
                        out=xst,
                        in_=xst_d[q:q + 2].rearrange("q p s c -> p q s c"))
                stage = stp.tile([_C, 2, 8, _C], bf16, tag="stage")
            j = c // 2
            half = 0 if (c % 2 == 0) else _K
            sidx = j if c % 2 == 0 else 4 + j
            cs = q * _T + j * _C
            if not wave:
                pb = ps2.tile([_C, 4, _T], f32,
                              tag="pbA" if wave_id % 2 == 0 else "pbB")
                pbf = pb[:, :, :].rearrange("p a b -> p (a b)")
            i = len(wave)
            nc.tensor.matmul(
                pbf[:, i * _T:i * _T + W],
                lhsT=e_sb[half:half + _K, cs:cs + _C],
                rhs=rhs_aug[half:half + _K, :],
                start=True, stop=True)
            wave.append((pbf, (q % 2, sidx), stage, xst, None))
            if len(wave) == 4:
                flush_wave()
            if c == 7:
                flush_wave()   # partial wave at pair end keeps stage complete
                if q % 2 == 1:
                    if "norelu" not in abl:
                        if "reludve" not in abl:
                            nc.scalar.activation(out=stage, in_=stage,
                                                 func=AF.Relu,
                                                 bias=0.0, scale=1.0)
                        else:
                            nc.vector.tensor_scalar_max(out=stage, in0=stage,
                                                        scalar1=0.0)
                    if "noout" not in abl:
                        # out ships on the Activation HWDGE ring to split
                        # dispatch load across both rings
                        nc.scalar.dma_start(
                            out=out_d[q - 1:q + 1].rearrange(
                                "q p s c -> p q s c"),
                            in_=stage)


def _host_fold(inputs):
    f64 = np.float64
    lin0 = np.asarray(inputs["lin0_w"], f64)
    conv1 = np.asarray(inputs["conv1_w"], f64)
    conv2 = np.asarray(inputs["conv2_w"], f64)
    lin1 = np.asarray(inputs["lin1_w"], f64)
    gamma = np.asarray(inputs["bn_gamma"], f64)
    beta = np.asarray(inputs["bn_beta"], f64)
    mean = np.asarray(inputs["bn_mean"], f64)
    var = np.asarray(inputs["bn_var"], f64)

    A = (lin0 @ conv1).astype(np.float32)                       # (64,128)
    scale = gamma / np.sqrt(var + _BN_EPS)
    shift = (beta - mean * scale).astype(np.float32)            # (128,)
    Bm = ((scale[:, None] * conv2) @ lin1).astype(np.float32)   # (128,64)
    return A, shift, Bm


def _shard_inputs(inputs):
    x = np.ascontiguousarray(np.asarray(inputs["x"], dtype=np.float32))
    A, shift, Bm = _host_fold(inputs)

    a_t = np.ascontiguousarray(A.T).astype(_BF16)   # (128, 64)
    b_t = np.ascontiguousarray(Bm.T)                # (64, 128) f32

    xf = x.reshape(_B, _C, _N)
    in_maps = []
    for g in range(_NCORES):
        b = g // _SLICES
        s = g % _SLICES
        x_sh = np.ascontiguousarray(xf[b, :, s * _NSH:(s + 1) * _NSH])
        xst_sh = x_sh.T + shift[None, :]            # (NSH, C) f32
        # tile layout: [pair, p, sidx, c] where n = pair*1024 + sidx*128 + p
        xst_t = np.ascontiguousarray(
            xst_sh.reshape(_NP, 8, _C, _C).transpose(0, 2, 1, 3))
        in_maps.append({
            "x": x_sh.astype(_BF16),
            "xst": xst_t.astype(_BF16),
            "a_t": a_t,
            "b_t": b_t,
        })
    return in_maps


def kernel(**inputs):
    global last_results
    import time
    from concourse.bass_utils import run_bass_kernel_spmd

    in_maps = _shard_inputs(inputs)
    nc = _build()
    last_err = None
    for attempt in range(3):
        try:
            last_results = run_bass_kernel_spmd(
                nc, in_maps, core_ids=list(range(_NCORES)))
            break
        except Exception as e:  # transient axon worker hiccups: retry
            last_err = e
            if attempt == 2:
                raise
            time.sleep(20.0 * (attempt + 1))

    full = np.empty((_B, _C, _N), np.float32)
    for g in range(_NCORES):
        b = g // _SLICES
        s = g % _SLICES
        o = last_results.results[g]["out"].astype(np.float32)
        # [pair, p, sidx, c] -> n-major (NSH, C) -> c-major
        o = o.transpose(0, 2, 1, 3).reshape(_NSH, _C)
        full[b, :, s * _NSH:(s + 1) * _NSH] = o.T
    return full.reshape(_B, _C, _D, _H, _W)
